# revision 1
# baseline (speedup 1.0000x reference)
"""ConvLogicTree layer for Trainium2 (8 NeuronCores, SPMD data-parallel over batch).

Math: the 16 soft binary gates are all affine in the monomial basis
[1, a, b, a*b], so softmax-gate-mixing per tree node collapses to
    node(a, b) = k0 + ka*a + kb*b + kab*(a*b)
with per-(channel, node) coefficients k = softmax(w) @ C  (C = gate->monomial
matrix).  Each output channel's 8 leaves are shifted 3x3-window views of 2
input channels; we materialize a 9-shift unfold U in DRAM scratch and pull the
1024 needed (shift, channel) rows per core with a single indexed dma_gather.
"""

import os
import sys

sys.path.insert(0, "/opt/trn_rl_repo")

import numpy as np

import concourse.bass as bass
import concourse.bacc as bacc
import concourse.mybir as mybir
import concourse.tile as tile
from contextlib import ExitStack
from concourse.bass_utils import run_bass_kernel_spmd
from concourse.library_config import mlp

F32 = mybir.dt.float32
I16 = mybir.dt.int16
AF = mybir.ActivationFunctionType
ALU = mybir.AluOpType

N_CORES = 8
B, C_IN, H, W = 16, 64, 32, 32
C_OUT = 128
NB = B // N_CORES          # batches per core
L = H * W                  # 1024 pixels
FD = NB * L                # free dim per compute op (batch-major pixels)
N_LEAF = 8

# gate g -> coefficients on [1, a, b, ab]
GATE_C = np.array(
    [
        [0, 0, 0, 0],    # 0
        [0, 0, 0, 1],    # ab
        [0, 1, 0, -1],   # a - ab
        [0, 1, 0, 0],    # a
        [0, 0, 1, -1],   # b - ab
        [0, 0, 1, 0],    # b
        [0, 1, 1, -2],   # a + b - 2ab
        [0, 1, 1, -1],   # a + b - ab
        [1, -1, -1, 1],  # 1 - (a+b-ab)
        [1, -1, -1, 2],  # 1 - (a+b-2ab)
        [1, 0, -1, 0],   # 1 - b
        [1, 0, -1, 1],   # 1 - b + ab
        [1, -1, 0, 0],   # 1 - a
        [1, -1, 0, 1],   # 1 - a + ab
        [1, 0, 0, -1],   # 1 - ab
        [1, 0, 0, 0],    # 1
    ],
    dtype=np.float32,
)

# tree wiring: (level, pair) -> weight row;  rows overlap across levels
# (faithful to the module: gate_idx = 2**level - 1 + pair)
L0_ROWS = [0, 1, 2, 3]
L1_ROWS = [1, 2]
L2_ROW = 3


def build_program():
    nc = bacc.Bacc("TRN2", target_bir_lowering=False, debug=False)

    x_in = nc.dram_tensor("x", [128, L], F32, kind="ExternalInput")
    w_in = nc.dram_tensor("w", [C_OUT, 7, 16], F32, kind="ExternalInput")
    cm_in = nc.dram_tensor("cmat", [128, 4, 7, 16], F32, kind="ExternalInput")
    gi_in = nc.dram_tensor("gidx", [128, 128], I16, kind="ExternalInput")
    out_ext = nc.dram_tensor("out", [NB, C_OUT, L], F32, kind="ExternalOutput")
    # 9-shift unfold scratch: row (s*128 + b*64 + c) holds shift-s of channel
    # c, batch b — full-128-partition writes straight from the xpad tile
    u_dram = nc.dram_tensor("u", [9 * NB * C_IN, L], F32)

    with tile.TileContext(nc) as tc, ExitStack() as ctx:
        pool = ctx.enter_context(tc.tile_pool(name="p", bufs=1))
        tmp = ctx.enter_context(tc.tile_pool(name="tmp", bufs=2))

        wt = pool.tile([128, 7, 16], F32)
        cm = pool.tile([128, 4, 7, 16], F32)
        en = pool.tile([128, 7, 16], F32)
        ssum = pool.tile([128, 7], F32)
        srec = pool.tile([128, 7], F32)
        km = pool.tile([128, 4, 7], F32)
        xp = pool.tile([128, 34 * 34], F32)
        gidx = pool.tile([128, 128], I16)
        lv = pool.tile([128, N_LEAF, FD], F32)
        nodes = [pool.tile([128, FD], F32, name=f"n{i}", tag=f"n{i}") for i in range(4)]
        mids = [pool.tile([128, FD], F32, name=f"m{i}", tag=f"m{i}") for i in range(2)]
        ot = pool.tile([128, FD], F32)

        nc.sync.dma_start(out=wt[:], in_=w_in[:])
        nc.sync.dma_start(out=cm[:], in_=cm_in[:])
        nc.sync.dma_start(out=gidx[:], in_=gi_in[:])

        # ---- softmax(w) @ C -> km[m, row]   (no max-subtraction: |w| ~ N(0,1))
        nc.scalar.activation(en[:], wt[:], AF.Exp)
        nc.vector.tensor_reduce(ssum[:], en[:], axis=mybir.AxisListType.X, op=ALU.add)
        nc.vector.reciprocal(srec[:], ssum[:])
        for n in range(7):
            nc.vector.tensor_scalar(
                en[:, n], en[:, n], srec[:, n : n + 1], None, op0=ALU.mult
            )
        for m in range(4):
            prd = tmp.tile([128, 7, 16], F32, tag="prd")
            nc.vector.tensor_tensor(prd[:], en[:], cm[:, m], op=ALU.mult)
            nc.vector.tensor_reduce(
                km[:, m], prd[:], axis=mybir.AxisListType.X, op=ALU.add
            )

        # ---- zero-padded input image per (b, c) partition
        nc.vector.memset(xp[:], 0.0)
        xpv = xp[:].rearrange("p (r c) -> p r c", r=34)
        nc.sync.dma_start(
            out=xpv[:, 1:33, 1:33],
            in_=x_in[:].rearrange("p (r c) -> p r c", r=32),
        )

        # ---- 9-shift unfold written to DRAM scratch; alternate between the
        # two HWDGE rings (SP / ACT) so the writes drain in parallel
        for s in range(9):
            ki, kj = s // 3, s % 3
            src = xpv[:, ki : ki + 32, kj : kj + 32]
            dst = u_dram[s * 128 : (s + 1) * 128, :]
            eng = nc.sync if s % 2 == 0 else nc.scalar
            eng.dma_start(out=dst, in_=src)

        # ---- derived coefficients for the factored node form
        #   node(a,b) = kab*(a + alpha)*(b + beta) + delta
        #   alpha = kb/kab, beta = ka/kab, delta = k0 - ka*kb/kab
        # (numerically safe here: |ka*kb/kab| stays tiny for softmax blends,
        #  verified against the host-side error proxy)
        alp = pool.tile([128, 7], F32)
        bet = pool.tile([128, 7], F32)
        dlt = pool.tile([128, 7], F32)
        rkab = pool.tile([128, 7], F32)
        nc.vector.reciprocal(rkab[:], km[:, 3])
        nc.vector.tensor_tensor(alp[:], km[:, 2], rkab[:], op=ALU.mult)
        nc.vector.tensor_tensor(bet[:], km[:, 1], rkab[:], op=ALU.mult)
        nc.vector.tensor_tensor(dlt[:], alp[:], km[:, 1], op=ALU.mult)
        nc.vector.tensor_tensor(dlt[:], km[:, 0], dlt[:], op=ALU.subtract)

        # ---- gather the 8 leaves, one call per level-0 pair (overlaps compute)
        nc.gpsimd.load_library(mlp)
        for p in range(4):
            nc.gpsimd.dma_gather(
                lv[:, 2 * p : 2 * p + 2].rearrange("p j (b f) -> p (j b) f", b=NB),
                u_dram[:],
                gidx[:, p * 32 : (p + 1) * 32],
                512,
                512,
                L,
            )

        # ---- tree:  node(a,b) = kab*(a+alpha)*(b+beta) + delta
        # engine split per node: Pool computes (b+beta), DVE the product via
        # scalar_tensor_tensor, ACT the final scale+shift (in-place).
        def emit_node(a_ap, b_ap, row, out_tile, t_eng):
            a_col = alp[:, row : row + 1]
            b_col = bet[:, row : row + 1]
            d_col = dlt[:, row : row + 1]
            kab = km[:, 3, row : row + 1]
            t = tmp.tile([128, FD], F32, tag="t")
            if t_eng == "dve":
                nc.vector.tensor_scalar(t[:], b_ap, b_col, None, op0=ALU.add)
            elif t_eng == "act":
                nc.scalar.activation(t[:], b_ap, AF.Identity, bias=b_col, scale=1.0)
            else:
                nc.gpsimd.tensor_scalar(t[:], b_ap, b_col, None, op0=ALU.add)
            nc.vector.scalar_tensor_tensor(
                out_tile[:], a_ap, a_col, t[:], op0=ALU.add, op1=ALU.mult
            )
            nc.scalar.activation(
                out_tile[:], out_tile[:], AF.Identity, bias=d_col, scale=kab
            )

        l0_eng = ["dve", "act", "dve", "act"]
        for p in range(4):
            emit_node(lv[:, 2 * p], lv[:, 2 * p + 1], L0_ROWS[p], nodes[p], l0_eng[p])
        emit_node(nodes[0][:], nodes[1][:], L1_ROWS[0], mids[0], "pool")
        emit_node(nodes[2][:], nodes[3][:], L1_ROWS[1], mids[1], "pool")
        emit_node(mids[0][:], mids[1][:], L2_ROW, ot, "pool")

        nc.sync.dma_start(
            out=out_ext[:].rearrange("b o f -> o b f"),
            in_=ot[:].rearrange("p (b f) -> p b f", b=NB),
        )

    nc.compile()
    return nc


def make_host_inputs(x, weights, leaf_indices):
    """Shared input prep: per-core in_maps (kernel shards batch over cores)."""
    x = np.ascontiguousarray(np.asarray(x), dtype=np.float32)
    weights = np.ascontiguousarray(np.asarray(weights), dtype=np.float32)
    leaf_indices = np.asarray(leaf_indices)

    feat = leaf_indices.astype(np.int64)          # [C_OUT, 8]
    c = feat // 9
    tap = feat % 9
    # U row = s*128 + b*64 + c ; gather order i = (j*NB + b)*128 + o
    order = np.zeros(2048, np.int16)
    for j in range(8):
        for b in range(NB):
            blk = j * NB + b
            order[blk * 128 : (blk + 1) * 128] = (
                tap[:, j] * 128 + b * C_IN + c[:, j]
            ).astype(np.int16)
    wrapped = np.zeros((16, 128), np.int16)
    ii = np.arange(2048)
    wrapped[ii % 16, ii // 16] = order[ii]
    gidx = np.tile(wrapped, (8, 1))               # replicated per Q7 core

    cmat = np.ascontiguousarray(
        np.broadcast_to(GATE_C.T.reshape(1, 4, 1, 16), (128, 4, 7, 16)),
        dtype=np.float32,
    )

    in_maps = []
    for core in range(N_CORES):
        xs = np.ascontiguousarray(
            x[core * NB : (core + 1) * NB].reshape(128, L)
        )
        in_maps.append({"x": xs, "w": weights, "cmat": cmat, "gidx": gidx})
    return in_maps


_NC_CACHE = {}


def kernel(x, weights, leaf_indices):
    key = "prog"
    if key not in _NC_CACHE:
        _NC_CACHE[key] = build_program()
    nc = _NC_CACHE[key]
    in_maps = make_host_inputs(x, weights, leaf_indices)
    res = run_bass_kernel_spmd(nc, in_maps, list(range(N_CORES)))
    out = np.concatenate(
        [r["out"].reshape(NB, C_OUT, H, W) for r in res.results], axis=0
    )
    return out



# revision 11
# speedup vs baseline: 2.4386x; 2.4386x over previous
"""ConvLogicTree layer for Trainium2 (8 NeuronCores, SPMD data-parallel over batch).

Math: the 16 soft binary gates are affine in the monomial basis [1, a, b, ab],
so softmax-gate-mixing per tree node collapses to
    node(a, b) = k0 + ka*a + kb*b + kab*(a*b) = kab*(a + kb/kab)*(b + ka/kab) + d.
Carrying each subtree as an affine-deferred product  v = G*u + D  (G, D per
output channel) turns every tree node into exactly
    u' = (u_a*p_a + q_a) * (u_b*p_b + q_b)
with per-channel scalars p, q chosen to keep |u| <= 1 (normalization absorbs
the softmax coefficients), plus one final affine  out = G_root*u_root + D_root.
That is 2 fp16 DVE tensor_scalar ops (4x perf mode) + 1 tensor_tensor (2x) per
node.  The whole pipeline runs in fp16 (tolerance 2e-2; measured ~4e-3).

Data movement: host pre-pads x to 34x34 fp16.  The device writes a 9-shift
unfold U[tap*64+c] = [b0 row | b1 row] (fp16, 2.25MB) to DRAM scratch, then one
indexed dma_gather per leaf-pair pulls 256 rows x 4KB into lv[o, slot, (b,pix)].
Merging both batches into one row halves the SWDGE descriptor count.
"""

import sys

sys.path.insert(0, "/opt/trn_rl_repo")

import numpy as np

import concourse.bass as bass
import concourse.bacc as bacc
import concourse.mybir as mybir
import concourse.tile as tile
from contextlib import ExitStack
from concourse.bass_utils import run_bass_kernel_spmd
from concourse.library_config import mlp

F32 = mybir.dt.float32
F16 = mybir.dt.float16
I16 = mybir.dt.int16
AF = mybir.ActivationFunctionType
ALU = mybir.AluOpType

N_CORES = 8
B, C_IN, H, W = 16, 64, 32, 32
C_OUT = 128
NB = B // N_CORES          # batches per core
L = H * W                  # 1024 pixels
FD = NB * L                # free dim per big compute op (batch-major pixels)
PW = 34                    # padded image width
PIMG = PW * PW             # 1156

# gate g -> coefficients on [1, a, b, ab]
GATE_C = np.array(
    [
        [0, 0, 0, 0],    # 0
        [0, 0, 0, 1],    # ab
        [0, 1, 0, -1],   # a - ab
        [0, 1, 0, 0],    # a
        [0, 0, 1, -1],   # b - ab
        [0, 0, 1, 0],    # b
        [0, 1, 1, -2],   # a + b - 2ab
        [0, 1, 1, -1],   # a + b - ab
        [1, -1, -1, 1],  # 1 - (a+b-ab)
        [1, -1, -1, 2],  # 1 - (a+b-2ab)
        [1, 0, -1, 0],   # 1 - b
        [1, 0, -1, 1],   # 1 - b + ab
        [1, -1, 0, 0],   # 1 - a
        [1, -1, 0, 1],   # 1 - a + ab
        [1, 0, 0, -1],   # 1 - ab
        [1, 0, 0, 0],    # 1
    ],
    dtype=np.float32,
)

# tree wiring (faithful to the module: gate_idx = 2**level - 1 + pair)
L0_ROWS = [0, 1, 2, 3]
L1_ROWS = [1, 2]
L2_ROW = 3


def build_program():
    nc = bacc.Bacc("TRN2", target_bir_lowering=False, debug=False)

    xp_in = nc.dram_tensor("xp", [128, PIMG], F16, kind="ExternalInput")
    w_in = nc.dram_tensor("w", [C_OUT, 7, 16], F32, kind="ExternalInput")
    cm_in = nc.dram_tensor("cmat", [128, 4, 7, 16], F32, kind="ExternalInput")
    gi_in = nc.dram_tensor("gidx", [128, 64], I16, kind="ExternalInput")
    out_ext = nc.dram_tensor("out", [C_OUT, FD], F16, kind="ExternalOutput")
    # unfold scratch: row ((tap*64 + c)*NB + b); the gather reads it as
    # [576, NB*1024] so one descriptor covers both batches of a (tap, c)
    u_dram = nc.dram_tensor("u", [9 * C_IN * NB, L], F16)

    with tile.TileContext(nc) as tc, ExitStack() as ctx:
        pool = ctx.enter_context(tc.tile_pool(name="p", bufs=1))
        tmp = ctx.enter_context(tc.tile_pool(name="tmp", bufs=2))

        wt = pool.tile([128, 7, 16], F32)
        cm = pool.tile([128, 4, 7, 16], F32)
        en = pool.tile([128, 7, 16], F32)
        ssum = pool.tile([128, 7], F32)
        srec = pool.tile([128, 7], F32)
        km = pool.tile([128, 4, 7], F32)
        xp = pool.tile([128, PIMG], F16)
        gidx = pool.tile([128, 64], I16)
        lv = pool.tile([128, 8, FD], F16)

        nc.sync.dma_start(out=xp[:], in_=xp_in[:])
        nc.scalar.dma_start(out=wt[:], in_=w_in[:])
        nc.scalar.dma_start(out=cm[:], in_=cm_in[:])
        nc.scalar.dma_start(out=gidx[:], in_=gi_in[:])

        nc.gpsimd.load_library(mlp)

        # ---- 9-shift unfold -> DRAM scratch.  xp partitions are c-major
        # (p = c*NB + b), so shift s writes the contiguous row block
        # [s*128, (s+1)*128) directly.
        xpv = xp[:].rearrange("p (r c) -> p r c", r=PW)
        for s in range(9):
            ki, kj = s // 3, s % 3
            src = xpv[:, ki : ki + 32, kj : kj + 32]
            dst = u_dram[s * 128 : (s + 1) * 128, :]
            eng = nc.sync if s % 2 == 0 else nc.scalar
            eng.dma_start(out=dst, in_=src)

        # ---- softmax(w) @ C -> km[m, row]  (no max-subtraction: |w| ~ N(0,1))
        nc.scalar.activation(en[:], wt[:], AF.Exp)
        nc.vector.tensor_reduce(ssum[:], en[:], axis=mybir.AxisListType.X, op=ALU.add)
        nc.vector.reciprocal(srec[:], ssum[:])
        for n in range(7):
            nc.vector.tensor_scalar(
                en[:, n], en[:, n], srec[:, n : n + 1], None, op0=ALU.mult
            )
        for m in range(4):
            prd = tmp.tile([128, 7, 16], F32, tag="prd")
            nc.vector.tensor_tensor(prd[:], en[:], cm[:, m], op=ALU.mult)
            nc.vector.tensor_reduce(
                km[:, m], prd[:], axis=mybir.AxisListType.X, op=ALU.add
            )

        # ---- per-node monomial coefficients
        #   alpha = kb/kab (with a), beta = ka/kab (with b),
        #   delta = k0 - ka*kb/kab, kappa = kab
        alp = pool.tile([128, 7], F32)
        bet = pool.tile([128, 7], F32)
        dlt = pool.tile([128, 7], F32)
        rkab = pool.tile([128, 7], F32)
        nc.vector.reciprocal(rkab[:], km[:, 3])
        nc.vector.tensor_tensor(alp[:], km[:, 2], rkab[:], op=ALU.mult)
        nc.vector.tensor_tensor(bet[:], km[:, 1], rkab[:], op=ALU.mult)
        nc.vector.tensor_tensor(dlt[:], alp[:], km[:, 1], op=ALU.mult)
        nc.vector.tensor_tensor(dlt[:], km[:, 0], dlt[:], op=ALU.subtract)

        # ---- normalized edge scalars.  Each tree edge turns child carry
        # (v = G*u + D) plus node offset tau into  (u*p + q)  with
        #   N = |G| + |D + tau|,  p = G/N,  q = (D + tau)/N
        # and the node's carry becomes G' = kappa * N_a * N_b, D' = delta.
        # pq[i] = (p_col, q_col) for edge i; edges: 8 leaf edges, 4 mid, 2 root.
        def edge_cols(G_col, Dtau_col, tag):
            # G_col: [128,1] or None (leaf: G=1); Dtau_col: [128,1]
            n_ = pool.tile([128, 1], F32, name=f"n{tag}", tag=f"n{tag}")
            rn = pool.tile([128, 1], F32, name=f"rn{tag}", tag=f"rn{tag}")
            q_ = pool.tile([128, 1], F32, name=f"qe{tag}", tag=f"qe{tag}")
            aq = tmp.tile([128, 1], F32, tag=f"aq{tag}")

            def fabs(dst, src):  # |x| = max(x, -x); abs_max is not valid ISA here
                ng = tmp.tile([128, 1], F32, tag=f"ng{tag}")
                nc.vector.tensor_scalar(ng[:], src, -1.0, None, op0=ALU.mult)
                nc.vector.tensor_tensor(dst, src, ng[:], op=ALU.max)

            fabs(aq[:], Dtau_col)
            if G_col is None:
                nc.vector.tensor_scalar(n_[:], aq[:], 1.0, None, op0=ALU.add)
            else:
                ag = tmp.tile([128, 1], F32, tag=f"ag{tag}")
                fabs(ag[:], G_col)
                nc.vector.tensor_tensor(n_[:], aq[:], ag[:], op=ALU.add)
            nc.vector.reciprocal(rn[:], n_[:])
            if G_col is None:
                p_ = rn
            else:
                p_ = pool.tile([128, 1], F32, name=f"pe{tag}", tag=f"pe{tag}")
                nc.vector.tensor_tensor(p_[:], G_col, rn[:], op=ALU.mult)
            nc.vector.tensor_tensor(q_[:], Dtau_col, rn[:], op=ALU.mult)
            return p_, q_, n_

        # L0 edges: children are leaves (G=1, D=0); tau = alpha/beta of row p
        l0_pq = []
        l0_G = []  # carry scale cols [128,1]
        for p in range(4):
            r = L0_ROWS[p]
            pa, qa, na = edge_cols(None, alp[:, r : r + 1], f"a0{p}")
            pb, qb, nb_ = edge_cols(None, bet[:, r : r + 1], f"b0{p}")
            g_ = pool.tile([128, 1], F32, name=f"g0{p}", tag=f"g0{p}")
            nc.vector.tensor_tensor(g_[:], na[:], nb_[:], op=ALU.mult)
            nc.vector.tensor_tensor(g_[:], g_[:], km[:, 3, r : r + 1], op=ALU.mult)
            l0_pq.append((pa, qa, pb, qb))
            l0_G.append(g_)

        # L1 edges: child n has carry (G=l0_G[n], D=delta[row of n])
        l1_pq = []
        l1_G = []
        for q in range(2):
            r = L1_ROWS[q]
            cols = []
            ns = []
            for side, child in enumerate((2 * q, 2 * q + 1)):
                rc = L0_ROWS[child]
                tau = alp[:, r : r + 1] if side == 0 else bet[:, r : r + 1]
                dt_ = tmp.tile([128, 1], F32, tag=f"dt1{q}{side}")
                nc.vector.tensor_tensor(
                    dt_[:], dlt[:, rc : rc + 1], tau, op=ALU.add
                )
                p_, q_, n_ = edge_cols(l0_G[child][:], dt_[:], f"e1{q}{side}")
                cols.append((p_, q_))
                ns.append(n_)
            g_ = pool.tile([128, 1], F32, name=f"g1{q}", tag=f"g1{q}")
            nc.vector.tensor_tensor(g_[:], ns[0][:], ns[1][:], op=ALU.mult)
            nc.vector.tensor_tensor(g_[:], g_[:], km[:, 3, r : r + 1], op=ALU.mult)
            l1_pq.append((cols[0][0], cols[0][1], cols[1][0], cols[1][1]))
            l1_G.append(g_)

        # root edges
        r = L2_ROW
        root_cols = []
        root_ns = []
        for side, q in enumerate((0, 1)):
            rc = L1_ROWS[q]
            tau = alp[:, r : r + 1] if side == 0 else bet[:, r : r + 1]
            dt_ = tmp.tile([128, 1], F32, tag=f"dtr{side}")
            nc.vector.tensor_tensor(dt_[:], dlt[:, rc : rc + 1], tau, op=ALU.add)
            p_, q_, n_ = edge_cols(l1_G[q][:], dt_[:], f"er{side}")
            root_cols.append((p_, q_))
            root_ns.append(n_)
        g_root = pool.tile([128, 1], F32, name="groot", tag="groot")
        nc.vector.tensor_tensor(g_root[:], root_ns[0][:], root_ns[1][:], op=ALU.mult)
        nc.vector.tensor_tensor(
            g_root[:], g_root[:], km[:, 3, r : r + 1], op=ALU.mult
        )

        # ---- gather the 8 leaf rows per channel, one call per leaf pair
        u_rows = u_dram[:].rearrange("(r two) f -> r (two f)", two=NB)
        for p in range(4):
            nc.gpsimd.dma_gather(
                lv[:, 2 * p : 2 * p + 2],
                u_rows,
                gidx[:, p * 16 : (p + 1) * 16],
                256,
                256,
                FD,
            )

        # ---- tree: u' = (ua*pa + qa) * (ub*pb + qb)
        nodes = [pool.tile([128, FD], F16, name=f"n{i}", tag=f"n{i}") for i in range(4)]
        mids = [pool.tile([128, FD], F16, name=f"m{i}", tag=f"m{i}") for i in range(2)]
        rt = pool.tile([128, FD], F16)
        ot = pool.tile([128, FD], F16)

        def emit_node(a_ap, b_ap, pq, out_tile, engines, tag):
            pa, qa, pb, qb = pq
            ea, eb = engines
            ah = tmp.tile([128, FD], F16, tag=f"ah{tag}")
            bh = tmp.tile([128, FD], F16, tag=f"bh{tag}")
            if ea == "act":
                nc.scalar.activation(
                    ah[:], a_ap, AF.Identity, bias=qa[:], scale=pa[:]
                )
            else:
                nc.vector.tensor_scalar(
                    ah[:], a_ap, pa[:], qa[:], op0=ALU.mult, op1=ALU.add
                )
            if eb == "act":
                nc.scalar.activation(
                    bh[:], b_ap, AF.Identity, bias=qb[:], scale=pb[:]
                )
            else:
                nc.vector.tensor_scalar(
                    bh[:], b_ap, pb[:], qb[:], op0=ALU.mult, op1=ALU.add
                )
            nc.vector.tensor_tensor(out_tile[:], ah[:], bh[:], op=ALU.mult)

        # early nodes lean on ACT (overlapped with gather transfers);
        # late/critical nodes stay on the faster fp16 DVE path
        l0_eng = [("act", "act"), ("act", "dve"), ("dve", "act"), ("dve", "dve")]
        for p in range(4):
            emit_node(
                lv[:, 2 * p], lv[:, 2 * p + 1], l0_pq[p], nodes[p], l0_eng[p], f"n{p}"
            )
        emit_node(nodes[0][:], nodes[1][:], l1_pq[0], mids[0], ("act", "dve"), "m0")
        emit_node(nodes[2][:], nodes[3][:], l1_pq[1], mids[1], ("dve", "dve"), "m1")
        emit_node(
            mids[0][:],
            mids[1][:],
            (root_cols[0][0], root_cols[0][1], root_cols[1][0], root_cols[1][1]),
            rt,
            ("dve", "dve"),
            "rt",
        )
        # final affine: out = G_root * u_root + delta(root row)
        nc.scalar.activation(
            ot[:], rt[:], AF.Identity, bias=dlt[:, r : r + 1], scale=g_root[:]
        )

        nc.sync.dma_start(out=out_ext[:], in_=ot[:])

    nc.compile()
    return nc


def make_host_inputs(x, weights, leaf_indices):
    """Per-core input maps: pad+cast x, build gather indices (layout prep only)."""
    x = np.asarray(x, dtype=np.float32)
    weights = np.ascontiguousarray(np.asarray(weights), dtype=np.float32)
    leaf_indices = np.asarray(leaf_indices)

    xpad = np.zeros((B, C_IN, PW, PW), np.float16)
    xpad[:, :, 1:33, 1:33] = x

    feat = leaf_indices.astype(np.int64)          # [C_OUT, 8]
    c = feat // 9
    tap = feat % 9
    # gather row = tap*64 + c ; call p covers slots (2p, 2p+1);
    # within a call, index position i = j_local*128 + o
    order = np.zeros(1024, np.int16)
    for j in range(8):
        p, j_local = j // 2, j % 2
        base = p * 256 + j_local * 128
        order[base : base + 128] = (tap[:, j] * C_IN + c[:, j]).astype(np.int16)
    wrapped = np.zeros((16, 64), np.int16)
    ii = np.arange(1024)
    wrapped[ii % 16, ii // 16] = order[ii]
    gidx = np.tile(wrapped, (8, 1))               # replicated per Q7 core

    cmat = np.ascontiguousarray(
        np.broadcast_to(GATE_C.T.reshape(1, 4, 1, 16), (128, 4, 7, 16)),
        dtype=np.float32,
    )

    in_maps = []
    for core in range(N_CORES):
        xs = np.ascontiguousarray(
            xpad[core * NB : (core + 1) * NB]
            .transpose(1, 0, 2, 3)        # c-major partitions: p = c*NB + b
            .reshape(C_IN * NB, PIMG)
        )
        in_maps.append({"xp": xs, "w": weights, "cmat": cmat, "gidx": gidx})
    return in_maps


def unpack_out(raw):
    """Device out [C_OUT, NB*L] fp16 -> [NB, C_OUT, H, W] f32."""
    return (
        np.asarray(raw, np.float32)
        .reshape(C_OUT, NB, H, W)
        .transpose(1, 0, 2, 3)
    )


_NC_CACHE = {}


def kernel(x, weights, leaf_indices):
    key = "prog"
    if key not in _NC_CACHE:
        _NC_CACHE[key] = build_program()
    nc = _NC_CACHE[key]
    in_maps = make_host_inputs(x, weights, leaf_indices)
    res = run_bass_kernel_spmd(nc, in_maps, list(range(N_CORES)))
    out = np.concatenate([unpack_out(r["out"]) for r in res.results], axis=0)
    return out


# revision 22
# speedup vs baseline: 3.4012x; 1.3947x over previous
"""ConvLogicTree layer for Trainium2 (8 NeuronCores, SPMD data-parallel over batch).

Math: the 16 soft binary gates are affine in the monomial basis [1, a, b, ab],
so softmax-gate-mixing per tree node collapses to
    node(a, b) = k0 + ka*a + kb*b + kab*(a*b) = kab*(a + kb/kab)*(b + ka/kab) + d.
Carrying each subtree as an affine-deferred product  v = G*u + D  (G, D per
output channel) turns every tree node into exactly
    u' = (u_a*p_a + q_a) * (u_b*p_b + q_b)
with per-channel scalars p, q chosen to keep |u| <= 1 (normalization absorbs
the softmax coefficients), plus one final affine  out = G_root*u_root + D_root.
That is 2 fp16 DVE tensor_scalar ops (4x perf mode) + 1 tensor_tensor (2x) per
node.  The whole pipeline runs in fp16 (tolerance 2e-2; measured ~4e-3).

Data movement: host pre-pads x to 34x34 fp16.  The device writes a 9-shift
unfold U[tap*64+c] = [b0 row | b1 row] (fp16, 2.25MB) to DRAM scratch, then one
indexed dma_gather per leaf-pair pulls 256 rows x 4KB into lv[o, slot, (b,pix)].
Merging both batches into one row halves the SWDGE descriptor count.
"""

import sys

sys.path.insert(0, "/opt/trn_rl_repo")

import numpy as np

import concourse.bass as bass
import concourse.bacc as bacc
import concourse.mybir as mybir
import concourse.tile as tile
from contextlib import ExitStack
from concourse.bass_utils import run_bass_kernel_spmd
from concourse.library_config import mlp

F32 = mybir.dt.float32
F16 = mybir.dt.float16
I16 = mybir.dt.int16
AF = mybir.ActivationFunctionType
ALU = mybir.AluOpType

N_CORES = 8
B, C_IN, H, W = 16, 64, 32, 32
C_OUT = 128
NB = B // N_CORES          # batches per core
L = H * W                  # 1024 pixels
FD = NB * L                # free dim per big compute op (batch-major pixels)
PW = 34                    # padded image width
PIMG = PW * PW             # 1156
SLAB = 31 * PW + 32        # contiguous span covering a 32x32 window (1086)
SLABV = 32 * PW            # view span, 34-divisible (1088; last 2 never read)
UROW = 1152                # slab rounded up so row stride is 256B-aligned
GROW = NB * UROW           # one gather row covers both batches (2304 elems)

# gate g -> coefficients on [1, a, b, ab]
GATE_C = np.array(
    [
        [0, 0, 0, 0],    # 0
        [0, 0, 0, 1],    # ab
        [0, 1, 0, -1],   # a - ab
        [0, 1, 0, 0],    # a
        [0, 0, 1, -1],   # b - ab
        [0, 0, 1, 0],    # b
        [0, 1, 1, -2],   # a + b - 2ab
        [0, 1, 1, -1],   # a + b - ab
        [1, -1, -1, 1],  # 1 - (a+b-ab)
        [1, -1, -1, 2],  # 1 - (a+b-2ab)
        [1, 0, -1, 0],   # 1 - b
        [1, 0, -1, 1],   # 1 - b + ab
        [1, -1, 0, 0],   # 1 - a
        [1, -1, 0, 1],   # 1 - a + ab
        [1, 0, 0, -1],   # 1 - ab
        [1, 0, 0, 0],    # 1
    ],
    dtype=np.float32,
)

# tree wiring (faithful to the module: gate_idx = 2**level - 1 + pair)
L0_ROWS = [0, 1, 2, 3]
L1_ROWS = [1, 2]
L2_ROW = 3


def build_program():
    nc = bacc.Bacc("TRN2", target_bir_lowering=False, debug=False)

    xp_in = nc.dram_tensor("xp", [128, PIMG], F16, kind="ExternalInput")
    w_in = nc.dram_tensor("w", [C_OUT, 7, 16], F32, kind="ExternalInput")
    cm_in = nc.dram_tensor("cmat", [128, 4, 7, 16], F32, kind="ExternalInput")
    gi_in = nc.dram_tensor("gidx", [128, 64], I16, kind="ExternalInput")
    out_ext = nc.dram_tensor("out", [C_OUT, FD], F16, kind="ExternalOutput")
    # unfold scratch: row ((tap*64 + c)*NB + b) holds the contiguous 1088-elem
    # slab xp[off : off+1088] (off = ki*34+kj) — one 2176B DMA chunk per
    # partition.  The gather reads it as [576, 2304] so one descriptor covers
    # both batches of a (tap, c); downstream ops view leaves as [b, 32, 34->32].
    u_dram = nc.dram_tensor("u", [9 * C_IN * NB, UROW], F16)

    with tile.TileContext(nc) as tc, ExitStack() as ctx:
        pool = ctx.enter_context(tc.tile_pool(name="p", bufs=1))
        tmp = ctx.enter_context(tc.tile_pool(name="tmp", bufs=2))

        wt = pool.tile([128, 7, 16], F32)
        cm = pool.tile([128, 4, 7, 16], F32)
        en = pool.tile([128, 7, 16], F32)
        ssum = pool.tile([128, 7], F32)
        srec = pool.tile([128, 7], F32)
        km = pool.tile([128, 4, 7], F32)
        xp = pool.tile([128, PIMG], F16)
        gidx = pool.tile([128, 64], I16)
        lv = pool.tile([128, 8, GROW], F16)

        nc.sync.dma_start(out=xp[:], in_=xp_in[:])
        nc.scalar.dma_start(out=wt[:], in_=w_in[:])
        nc.scalar.dma_start(out=cm[:], in_=cm_in[:])
        nc.scalar.dma_start(out=gidx[:], in_=gi_in[:])

        nc.gpsimd.load_library(mlp)

        # zero-fill the 66-elem row tails the unfold below never writes (the
        # gather reads whole 1152-elem rows; the tails reach lv but no compute
        # op ever reads them — this just keeps uninitialized-memory checks calm)
        zt = pool.tile([128, 9 * (UROW - SLAB)], F16)
        nc.vector.memset(zt[:], 0.0)
        nc.scalar.dma_start(
            out=u_dram[:].rearrange("(s p) e -> p s e", s=9)[:, :, SLAB:UROW],
            in_=zt[:].rearrange("p (s e) -> p s e", s=9),
        )

        # ---- 9-shift unfold -> DRAM scratch.  xp partitions are c-major
        # (p = c*NB + b), so shift s writes the contiguous row block
        # [s*128, (s+1)*128) directly; src is one contiguous slab per partition.
        for s in range(9):
            ki, kj = s // 3, s % 3
            off = ki * PW + kj
            src = xp[:, off : off + SLAB]
            dst = u_dram[s * 128 : (s + 1) * 128, 0:SLAB]
            eng = nc.sync if s % 2 == 0 else nc.scalar
            eng.dma_start(out=dst, in_=src)

        # ---- softmax(w) @ C -> km[m, row]  (no max-subtraction: |w| ~ N(0,1))
        nc.scalar.activation(en[:], wt[:], AF.Exp)
        nc.vector.tensor_reduce(ssum[:], en[:], axis=mybir.AxisListType.X, op=ALU.add)
        nc.vector.reciprocal(srec[:], ssum[:])
        for n in range(7):
            nc.vector.tensor_scalar(
                en[:, n], en[:, n], srec[:, n : n + 1], None, op0=ALU.mult
            )
        for m in range(4):
            prd = tmp.tile([128, 7, 16], F32, tag="prd")
            nc.vector.tensor_tensor(prd[:], en[:], cm[:, m], op=ALU.mult)
            nc.vector.tensor_reduce(
                km[:, m], prd[:], axis=mybir.AxisListType.X, op=ALU.add
            )

        # ---- per-node monomial coefficients
        #   alpha = kb/kab (with a), beta = ka/kab (with b),
        #   delta = k0 - ka*kb/kab, kappa = kab
        alp = pool.tile([128, 7], F32)
        bet = pool.tile([128, 7], F32)
        dlt = pool.tile([128, 7], F32)
        rkab = pool.tile([128, 7], F32)
        nc.vector.reciprocal(rkab[:], km[:, 3])
        nc.vector.tensor_tensor(alp[:], km[:, 2], rkab[:], op=ALU.mult)
        nc.vector.tensor_tensor(bet[:], km[:, 1], rkab[:], op=ALU.mult)
        nc.vector.tensor_tensor(dlt[:], alp[:], km[:, 1], op=ALU.mult)
        nc.vector.tensor_tensor(dlt[:], km[:, 0], dlt[:], op=ALU.subtract)

        # ---- normalized edge scalars.  Each tree edge turns child carry
        # (v = G*u + D) plus node offset tau into  (u*p + q)  with
        #   N = |G| + |D + tau|,  p = G/N,  q = (D + tau)/N
        # and the node's carry becomes G' = kappa * N_a * N_b, D' = delta.
        # pq[i] = (p_col, q_col) for edge i; edges: 8 leaf edges, 4 mid, 2 root.
        def edge_cols(G_col, Dtau_col, tag):
            # G_col: [128,1] or None (leaf: G=1); Dtau_col: [128,1]
            n_ = pool.tile([128, 1], F32, name=f"n{tag}", tag=f"n{tag}")
            rn = pool.tile([128, 1], F32, name=f"rn{tag}", tag=f"rn{tag}")
            q_ = pool.tile([128, 1], F32, name=f"qe{tag}", tag=f"qe{tag}")
            aq = tmp.tile([128, 1], F32, tag=f"aq{tag}")

            def fabs(dst, src):  # |x| = max(x, -x); abs_max is not valid ISA here
                ng = tmp.tile([128, 1], F32, tag=f"ng{tag}")
                nc.vector.tensor_scalar(ng[:], src, -1.0, None, op0=ALU.mult)
                nc.vector.tensor_tensor(dst, src, ng[:], op=ALU.max)

            fabs(aq[:], Dtau_col)
            if G_col is None:
                nc.vector.tensor_scalar(n_[:], aq[:], 1.0, None, op0=ALU.add)
            else:
                ag = tmp.tile([128, 1], F32, tag=f"ag{tag}")
                fabs(ag[:], G_col)
                nc.vector.tensor_tensor(n_[:], aq[:], ag[:], op=ALU.add)
            nc.vector.reciprocal(rn[:], n_[:])
            if G_col is None:
                p_ = rn
            else:
                p_ = pool.tile([128, 1], F32, name=f"pe{tag}", tag=f"pe{tag}")
                nc.vector.tensor_tensor(p_[:], G_col, rn[:], op=ALU.mult)
            nc.vector.tensor_tensor(q_[:], Dtau_col, rn[:], op=ALU.mult)
            return p_, q_, n_

        # L0 edges: children are leaves (G=1, D=0); tau = alpha/beta of row p
        l0_pq = []
        l0_G = []  # carry scale cols [128,1]
        for p in range(4):
            r = L0_ROWS[p]
            pa, qa, na = edge_cols(None, alp[:, r : r + 1], f"a0{p}")
            pb, qb, nb_ = edge_cols(None, bet[:, r : r + 1], f"b0{p}")
            g_ = pool.tile([128, 1], F32, name=f"g0{p}", tag=f"g0{p}")
            nc.vector.tensor_tensor(g_[:], na[:], nb_[:], op=ALU.mult)
            nc.vector.tensor_tensor(g_[:], g_[:], km[:, 3, r : r + 1], op=ALU.mult)
            l0_pq.append((pa, qa, pb, qb))
            l0_G.append(g_)

        # L1 edges: child n has carry (G=l0_G[n], D=delta[row of n])
        l1_pq = []
        l1_G = []
        for q in range(2):
            r = L1_ROWS[q]
            cols = []
            ns = []
            for side, child in enumerate((2 * q, 2 * q + 1)):
                rc = L0_ROWS[child]
                tau = alp[:, r : r + 1] if side == 0 else bet[:, r : r + 1]
                dt_ = tmp.tile([128, 1], F32, tag=f"dt1{q}{side}")
                nc.vector.tensor_tensor(
                    dt_[:], dlt[:, rc : rc + 1], tau, op=ALU.add
                )
                p_, q_, n_ = edge_cols(l0_G[child][:], dt_[:], f"e1{q}{side}")
                cols.append((p_, q_))
                ns.append(n_)
            g_ = pool.tile([128, 1], F32, name=f"g1{q}", tag=f"g1{q}")
            nc.vector.tensor_tensor(g_[:], ns[0][:], ns[1][:], op=ALU.mult)
            nc.vector.tensor_tensor(g_[:], g_[:], km[:, 3, r : r + 1], op=ALU.mult)
            l1_pq.append((cols[0][0], cols[0][1], cols[1][0], cols[1][1]))
            l1_G.append(g_)

        # root edges
        r = L2_ROW
        root_cols = []
        root_ns = []
        for side, q in enumerate((0, 1)):
            rc = L1_ROWS[q]
            tau = alp[:, r : r + 1] if side == 0 else bet[:, r : r + 1]
            dt_ = tmp.tile([128, 1], F32, tag=f"dtr{side}")
            nc.vector.tensor_tensor(dt_[:], dlt[:, rc : rc + 1], tau, op=ALU.add)
            p_, q_, n_ = edge_cols(l1_G[q][:], dt_[:], f"er{side}")
            root_cols.append((p_, q_))
            root_ns.append(n_)
        g_root = pool.tile([128, 1], F32, name="groot", tag="groot")
        nc.vector.tensor_tensor(g_root[:], root_ns[0][:], root_ns[1][:], op=ALU.mult)
        nc.vector.tensor_tensor(
            g_root[:], g_root[:], km[:, 3, r : r + 1], op=ALU.mult
        )

        # ---- gather the 8 leaf rows per channel, one call per leaf pair
        u_rows = u_dram[:].rearrange("(r two) f -> r (two f)", two=NB)
        for p in range(4):
            nc.gpsimd.dma_gather(
                lv[:, 2 * p : 2 * p + 2],
                u_rows,
                gidx[:, p * 16 : (p + 1) * 16],
                256,
                256,
                GROW,
            )

        # ---- tree: u' = (ua*pa + qa) * (ub*pb + qb)
        nodes = [pool.tile([128, FD], F16, name=f"n{i}", tag=f"n{i}") for i in range(4)]
        mids = [pool.tile([128, FD], F16, name=f"m{i}", tag=f"m{i}") for i in range(2)]
        rt = pool.tile([128, FD], F16)
        ot = pool.tile([128, FD], F16)

        def leaf_view(j):
            # [128, b, 32, 32] strided window into the gathered 1152-elem slabs
            return (
                lv[:, j]
                .rearrange("p (b q) -> p b q", b=NB)[:, :, 0:SLABV]
                .rearrange("p b (r c) -> p b r c", c=PW)[:, :, :, 0:32]
            )

        def win_dst(t):
            # packed [128, FD] tile shaped to match a leaf_view operand
            return t[:].rearrange("p (b r c) -> p b r c", b=NB, r=32)

        def emit_node(a_ap, b_ap, pq, out_tile, engines, tag):
            pa, qa, pb, qb = pq
            ea, eb = engines
            ah = tmp.tile([128, FD], F16, tag=f"ah{tag}")
            bh = tmp.tile([128, FD], F16, tag=f"bh{tag}")
            dst_a = ah[:] if len(a_ap.shape) == 2 else win_dst(ah)
            dst_b = bh[:] if len(b_ap.shape) == 2 else win_dst(bh)
            if ea == "act":
                nc.scalar.activation(
                    dst_a, a_ap, AF.Identity, bias=qa[:], scale=pa[:]
                )
            else:
                nc.vector.tensor_scalar(
                    dst_a, a_ap, pa[:], qa[:], op0=ALU.mult, op1=ALU.add
                )
            if eb == "act":
                nc.scalar.activation(
                    dst_b, b_ap, AF.Identity, bias=qb[:], scale=pb[:]
                )
            else:
                nc.vector.tensor_scalar(
                    dst_b, b_ap, pb[:], qb[:], op0=ALU.mult, op1=ALU.add
                )
            nc.vector.tensor_tensor(out_tile[:], ah[:], bh[:], op=ALU.mult)

        # early nodes lean on ACT (overlapped with gather transfers);
        # late/critical nodes stay on the faster fp16 DVE path
        l0_eng = [("act", "act"), ("act", "dve"), ("dve", "act"), ("dve", "dve")]
        for p in range(4):
            emit_node(
                leaf_view(2 * p),
                leaf_view(2 * p + 1),
                l0_pq[p],
                nodes[p],
                l0_eng[p],
                f"n{p}",
            )
        emit_node(nodes[0][:], nodes[1][:], l1_pq[0], mids[0], ("act", "dve"), "m0")
        emit_node(nodes[2][:], nodes[3][:], l1_pq[1], mids[1], ("dve", "dve"), "m1")
        emit_node(
            mids[0][:],
            mids[1][:],
            (root_cols[0][0], root_cols[0][1], root_cols[1][0], root_cols[1][1]),
            rt,
            ("dve", "dve"),
            "rt",
        )
        # final affine: out = G_root * u_root + delta(root row)
        nc.scalar.activation(
            ot[:], rt[:], AF.Identity, bias=dlt[:, r : r + 1], scale=g_root[:]
        )

        nc.sync.dma_start(out=out_ext[:], in_=ot[:])

    nc.compile()
    return nc


def make_host_inputs(x, weights, leaf_indices):
    """Per-core input maps: pad+cast x, build gather indices (layout prep only)."""
    x = np.asarray(x, dtype=np.float32)
    weights = np.ascontiguousarray(np.asarray(weights), dtype=np.float32)
    leaf_indices = np.asarray(leaf_indices)

    xpad = np.zeros((B, C_IN, PW, PW), np.float16)
    xpad[:, :, 1:33, 1:33] = x

    feat = leaf_indices.astype(np.int64)          # [C_OUT, 8]
    c = feat // 9
    tap = feat % 9
    # gather row = tap*64 + c ; call p covers slots (2p, 2p+1);
    # within a call, index position i = j_local*128 + o
    order = np.zeros(1024, np.int16)
    for j in range(8):
        p, j_local = j // 2, j % 2
        base = p * 256 + j_local * 128
        order[base : base + 128] = (tap[:, j] * C_IN + c[:, j]).astype(np.int16)
    wrapped = np.zeros((16, 64), np.int16)
    ii = np.arange(1024)
    wrapped[ii % 16, ii // 16] = order[ii]
    gidx = np.tile(wrapped, (8, 1))               # replicated per Q7 core

    cmat = np.ascontiguousarray(
        np.broadcast_to(GATE_C.T.reshape(1, 4, 1, 16), (128, 4, 7, 16)),
        dtype=np.float32,
    )

    in_maps = []
    for core in range(N_CORES):
        xs = np.ascontiguousarray(
            xpad[core * NB : (core + 1) * NB]
            .transpose(1, 0, 2, 3)        # c-major partitions: p = c*NB + b
            .reshape(C_IN * NB, PIMG)
        )
        in_maps.append({"xp": xs, "w": weights, "cmat": cmat, "gidx": gidx})
    return in_maps


def unpack_out(raw):
    """Device out [C_OUT, NB*L] fp16 -> [NB, C_OUT, H, W] f32."""
    return (
        np.asarray(raw, np.float32)
        .reshape(C_OUT, NB, H, W)
        .transpose(1, 0, 2, 3)
    )


_NC_CACHE = {}


def kernel(x, weights, leaf_indices):
    key = "prog"
    if key not in _NC_CACHE:
        _NC_CACHE[key] = build_program()
    nc = _NC_CACHE[key]
    in_maps = make_host_inputs(x, weights, leaf_indices)
    res = run_bass_kernel_spmd(nc, in_maps, list(range(N_CORES)))
    out = np.concatenate([unpack_out(r["out"]) for r in res.results], axis=0)
    return out
